# revision 2
# baseline (speedup 1.0000x reference)
"""BinaryMoSLinear Trainium2 kernel, fp8-DoubleRow edition (8-core SPMD, DP over tokens).

Math (per reference):
    routing = softmax(xf @ gate_w.T);  in_s = routing @ ics;  out_s = routing @ ocs
    out = (xf * in_s) @ sign(weight).T * out_s + bias

Device factorization (per core: 1024 tokens, full weight):
    expT[e,t] = exp(logitsT[e,t]) raw; den = sum_e expT; is_raw = ics^T expT
    a[h,t] = xT * is_raw   (softmax denominators factored out, applied at the end)
    main[t,o] = a^T sign(w)^T;  out = main * os_raw / den^2 + bias

fp8 path: a is stored as an e4m3 hi/lo pair (aHi + aLo ~ bf16 accuracy); sign(w)
is exact in e4m3.  Mains run in DoubleRow perf mode: each matmul contracts TWO
128-deep h-chunks (stationary [128,2,128], moving [128,2,512]) at 0.5 cyc/row —
2x the bf16 rate for hi+lo combined.  The stationary (an a-pair) is reused
across G=2 o-chunks to amortize ldweights.
"""

import numpy as np

import concourse.bass as bass
import concourse.mybir as mybir
from concourse import tile
from concourse.bass_utils import run_bass_kernel_spmd
from concourse.masks import make_identity

F32 = mybir.dt.float32
BF16 = mybir.dt.bfloat16
FP8 = mybir.dt.float8e4
DR = mybir.MatmulPerfMode.DoubleRow
AF = mybir.ActivationFunctionType
ALU = mybir.AluOpType

P = 128
E = 8
N_CORES = 8

FULL_B, FULL_S, FULL_H, FULL_O = 4, 2048, 4096, 4096
TOK = FULL_B * FULL_S // N_CORES  # 1024 tokens per core

ON = 512      # psum / main moving width per o-chunk
G = 2         # o-chunks sharing one stationary load (wbt group width = G*ON)
WBT_BUFS = 4  # wbt stage ring


# --------------------------------------------------------------------------
# This container's walrus build accepts at most ONE sync-wait command per
# instruction.  Tile's scheduler freely stacks several waits on one
# instruction, so rewrite the BIR JSON before compile: excess waits become
# single-wait NoOps immediately preceding the instruction on the same engine.
_MAXW = 1


def _split_excess_waits(bir_json: bytes, maxw: int = _MAXW) -> bytes:
    import json as _json

    j = _json.loads(bir_json)
    ctr = 0
    for fn in j["functions"]:
        for blk in fn["blocks"]:
            new = []
            for inst in blk["instructions"]:
                si = inst.get("sync_info")
                if si:
                    waits = si.get("on_wait") or []
                    if len(waits) > maxw:
                        extra, keep = waits[:-maxw], waits[-maxw:]
                        for i in range(0, len(extra), maxw):
                            ctr += 1
                            nop = {
                                "name": f"I-wsplit-{ctr}",
                                "opcode": "NoOp",
                                "engine": inst["engine"],
                                "ins": [],
                                "outs": [],
                                "sync_info": {
                                    "on_wait": extra[i : i + maxw],
                                    "on_update": [],
                                },
                            }
                            if "debug" in inst:
                                nop["debug"] = inst["debug"]
                            new.append(nop)
                        si["on_wait"] = keep
                new.append(inst)
            blk["instructions"] = new
    return _json.dumps(j).encode()


def _install_wait_split():
    from concourse import bass2jax, bass_utils

    orig = bass_utils.compile_bir_kernel
    if getattr(orig, "_wait_split_wrapped", False):
        return

    def wrapped(bir_json, tmpdir, neff_name="file.neff"):
        return orig(_split_excess_waits(bir_json), tmpdir, neff_name)

    wrapped._wait_split_wrapped = True
    bass_utils.compile_bir_kernel = wrapped
    bass2jax.compile_bir_kernel = wrapped


_install_wait_split()
# --------------------------------------------------------------------------


def build_nc(tok=TOK, h=FULL_H, o=FULL_O):
    HC = h // P              # h chunks
    HP = HC // 2             # h chunk pairs (DoubleRow k-tiles)
    TB = tok // P            # token blocks
    THW = min(512, tok)      # gating token-half width
    TH = tok // THW
    OC = o // ON             # o chunks
    OG = OC // G             # o-chunk groups (stationary reuse)
    JH = min(16, HC)         # h-chunks per wbt stage
    HH = HC // JH            # stages per group
    NSG = G * ON // P        # o-strips per stage
    GON = G * ON
    assert tok % P == 0 and h % (2 * P) == 0 and o % GON == 0 and HC % JH == 0

    nc = bass.Bass("TRN2", target_bir_lowering=False, debug=False,
                   num_devices=N_CORES)

    x_d = nc.declare_dram_parameter("x", [tok, h], F32, isOutput=False)
    w_d = nc.declare_dram_parameter("weight", [o, h], F32, isOutput=False)
    b_d = nc.declare_dram_parameter("bias", [o], F32, isOutput=False)
    gw_d = nc.declare_dram_parameter("gate_w", [E, h], F32, isOutput=False)
    ics_d = nc.declare_dram_parameter("ics", [E, h], F32, isOutput=False)
    ocs_d = nc.declare_dram_parameter("ocs", [E, o], F32, isOutput=False)
    out_d = nc.declare_dram_parameter("out", [tok, o], F32, isOutput=True)

    with tile.TileContext(nc) as tc:
        with (
            tc.tile_pool(name="const", bufs=1) as const,
            tc.tile_pool(name="sb", bufs=2) as sb,
            tc.tile_pool(name="wsgn", bufs=9) as wsgnp,
            tc.tile_pool(name="wbt", bufs=WBT_BUFS) as wbtp,
            tc.tile_pool(name="tring", bufs=2) as tringp,
            tc.tile_pool(name="pmm", bufs=4, space="PSUM") as pmm,
            tc.tile_pool(name="pTx", bufs=2, space="PSUM") as pTx,
            tc.tile_pool(name="pTw", bufs=1, space="PSUM") as pTw,
        ):
            # ---- early DMA: first x strips + weight stage 0 ----
            x_bf_tiles = {}
            CH = min(2048, h)
            x_bf_tiles[0] = sb.tile([P, h], BF16, tag="xbf", name="xbf_pre0")
            for c0 in range(0, h, CH):
                xpre = sb.tile([P, JH * P], F32, tag="wf32",
                               name=f"xpre_{c0}")
                nc.sync.dma_start(
                    out=xpre[:, 0:CH], in_=x_d[0:P, c0 : c0 + CH]
                )
                nc.vector.tensor_copy(
                    out=x_bf_tiles[0][:, c0 : c0 + CH], in_=xpre[:, 0:CH]
                )
            aux1 = const.tile([P, max(h, o)], BF16, name="aux1")
            nc.gpsimd.dma_start(out=aux1[0:E, 0:h], in_=gw_d[:, :])
            if TB > 1:
                x_bf_tiles[1] = sb.tile([P, h], BF16, tag="xbf",
                                        name="xbf_pre1")
                nc.gpsimd.dma_start(out=x_bf_tiles[1], in_=x_d[P : 2 * P, :])

            def stage_load(og, hh):
                """DMA + sign one weight stage: o-cols [og*GON, (og+1)*GON),
                h-cols [hh*JH*P, (hh+1)*JH*P). Returns fp8-signed strips."""
                o0 = og * GON
                wsgn = []
                for st in range(NSG):
                    wf = sb.tile([P, JH * P], F32, tag="wf32",
                                 name=f"wf_{og}_{hh}_{st}")
                    nc.sync.dma_start(
                        out=wf,
                        in_=w_d[
                            o0 + st * P : o0 + (st + 1) * P,
                            hh * JH * P : (hh + 1) * JH * P,
                        ],
                    )
                    ws = wsgnp.tile([P, JH * P], FP8, tag="wsgn",
                                    name=f"ws_{og}_{hh}_{st}")
                    nc.scalar.activation(ws, wf, AF.Sign)
                    wsgn.append(ws)
                return wsgn

            wsgn0 = [stage_load(0, hh) for hh in range(HH)]

            # ---- constants ----
            id_bf = const.tile([P, P], BF16, name="id_bf")
            make_identity(nc, id_bf)
            id8 = const.tile([P, P], FP8, name="id8")
            make_identity(nc, id8)
            ones_bf = const.tile([P, 1], BF16, name="ones_bf")
            nc.vector.memset(ones_bf, 1.0)
            ones_e = const.tile([P, E], F32, name="ones_e")
            nc.vector.memset(ones_e, 1.0)

            # aT holds bf16 xT, then (after the in_scale fold) its bytes are
            # reused in place: even bytes = fp8 hi, odd bytes = fp8 lo.
            aT = const.tile([P, HC * tok], BF16, name="aT")
            aT3 = aT.rearrange("p (hc t) -> p hc t", t=tok)
            _ab = aT.bitcast(FP8).rearrange("p (n two) -> p n two", two=2)
            xHi = _ab[:, :, 0]
            xLo = _ab[:, :, 1]
            xHi3 = xHi.rearrange("p (hc t) -> p hc t", t=tok)
            xLo3 = xLo.rearrange("p (hc t) -> p hc t", t=tok)
            xHi4 = xHi.rearrange("p (hp two t) -> p hp two t", two=2, t=tok)
            xLo4 = xLo.rearrange("p (hp two t) -> p hp two t", two=2, t=tok)

            expT = const.tile([P, tok], BF16, name="expT")
            nc.vector.memset(expT, 0.0)

            gwT = const.tile([P, HC * E], BF16, name="gwT")
            # aux1: holds gate_w for the gwT build, then is overwritten with
            # the broadcast bias for the epilogue.  aux2: in_channel_scale
            # rows during phase A, then out_channel_scale rows for phase C
            # (rows E.. are never read through a nonzero stationary).
            aux2 = const.tile([P, max(h, o)], BF16, name="aux2")
            nc.vector.memset(aux2, 0.0)
            nc.gpsimd.dma_start(out=aux2[0:E, 0:h], in_=ics_d[:, :])
            gw_bf = aux1
            ics_bf = aux2

            # gwT: transpose gate_w (E rows live, zero-padded)
            for hc in range(HC):
                pt = pTx.tile([P, min(8, HC) * P], BF16, tag="Tx",
                              name=f"ptg_{hc}")
                nc.tensor.transpose(
                    pt[:, 0:P], gw_bf[:, hc * P : (hc + 1) * P], id_bf
                )
                nc.vector.tensor_copy(
                    out=gwT[:, hc * E : (hc + 1) * E], in_=pt[:, 0:E]
                )
            bias_bc = aux1
            nc.gpsimd.dma_start(
                out=bias_bc[:, 0:o], in_=b_d[None, :].to_broadcast((P, o))
            )

            # ---- phase A1: transpose x into fp8 hi/lo ----
            for tb in range(TB):
                t0 = tb * P
                if tb in x_bf_tiles:
                    x_bf = x_bf_tiles.pop(tb)
                else:
                    x_bf = sb.tile([P, h], BF16, tag="xbf")
                    nc.gpsimd.dma_start(out=x_bf, in_=x_d[t0 : t0 + P, :])
                XG = min(8, HC)
                for gi, j0 in enumerate(range(0, HC, XG)):
                    pt = pTx.tile([P, XG * P], BF16, tag="Tx")
                    for k in range(XG):
                        nc.tensor.transpose(
                            pt[:, k * P : (k + 1) * P],
                            x_bf[:, (j0 + k) * P : (j0 + k + 1) * P],
                            id_bf,
                        )
                    ptv = pt.rearrange("p (b t) -> p b t", t=P)
                    dst = aT3[:, j0 : j0 + XG, t0 : t0 + P]
                    if gi % 2 == 0:
                        nc.scalar.activation(dst, ptv, AF.Copy)
                    else:
                        nc.vector.tensor_copy(out=dst, in_=ptv)

            # ---- phase A2: gating + fold + hi/lo re-split, per token half ----
            for th in range(TH):
                s0 = th * THW
                pl = pmm.tile([P, ON], F32, tag="mm", name=f"pl_{th}")
                for hc in range(HC):
                    nc.tensor.matmul(
                        pl[0:E, 0:THW],
                        gwT[:, hc * E : (hc + 1) * E],
                        aT3[:, hc, s0 : s0 + THW],
                        start=(hc == 0),
                        stop=(hc == HC - 1),
                    )
                nc.scalar.activation(
                    expT[0:E, s0 : s0 + THW], pl[0:E, 0:THW], AF.Exp
                )
                pd = pmm.tile([P, ON], F32, tag="mm")
                nc.tensor.matmul(
                    pd[0:1, 0:THW], ones_bf, expT[:, s0 : s0 + THW],
                    start=True, stop=True,
                )
                idr = sb.tile([P, ON], F32, tag="out", bufs=2,
                              name=f"idr_{th}")
                nc.vector.reciprocal(idr[0:1, 0:THW], pd[0:1, 0:THW])
                pde = pmm.tile([P, ON], F32, tag="mm")
                nc.tensor.matmul(
                    pde[0:E, 0:THW], ones_e[0:1, 0:E], idr[0:1, 0:THW],
                    start=True, stop=True,
                )
                nc.vector.tensor_tensor(
                    expT[0:E, s0 : s0 + THW],
                    expT[0:E, s0 : s0 + THW],
                    pde[0:E, 0:THW],
                    ALU.mult,
                )
                for hc in range(HC):
                    pis = pmm.tile([P, ON], F32, tag="mm")
                    nc.tensor.matmul(
                        pis[:, 0:THW],
                        ics_bf[:, hc * P : (hc + 1) * P],
                        expT[:, s0 : s0 + THW],
                        start=True,
                        stop=True,
                    )
                    tmp = tringp.tile([P, THW], BF16, tag="t")
                    nc.vector.tensor_tensor(
                        tmp, aT3[:, hc, s0 : s0 + THW], pis[:, 0:THW],
                        ALU.mult,
                    )
                    nc.scalar.activation(
                        xHi3[:, hc, s0 : s0 + THW], tmp, AF.Copy
                    )
                    nc.vector.tensor_tensor(
                        xLo3[:, hc, s0 : s0 + THW],
                        tmp,
                        xHi3[:, hc, s0 : s0 + THW],
                        ALU.subtract,
                    )

            ocs_bf = aux2
            nc.gpsimd.dma_start(out=ocs_bf[0:E, 0:o], in_=ocs_d[:, :])

            # ---- phase C: weight transpose stages + DoubleRow mains ----
            def stage_transpose_ops(og, hh, wsgn):
                """Thunks: each transposes 4 o-strip blocks of one h-chunk j
                into the wbt stage tile (fp8, stride-2 psum staging)."""
                wt = wbtp.tile([P, JH * GON], FP8, tag="wbt",
                               name=f"wt_{og}_{hh}")

                def make(j):
                    def emit():
                        pt8 = pTw.tile([P, NSG * P * 2], FP8, tag="Tw",
                                       name=f"ptw_{og}_{hh}_{j}")
                        p3 = pt8.rearrange("p (b t two) -> p b t two",
                                           b=NSG, t=P)
                        for st in range(NSG):
                            nc.tensor.transpose(
                                p3[:, st, :, 0],
                                wsgn[st][:, j * P : (j + 1) * P],
                                id8,
                            )
                        eng = nc.vector if j % 2 == 0 else nc.scalar
                        dst = wt.rearrange("p (jj s) -> p jj s", s=GON)[
                            :, j, :
                        ].rearrange("p (b t) -> p b t", t=P)
                        if j % 2 == 0:
                            nc.vector.tensor_copy(out=dst, in_=p3[:, :, :, 0])
                        else:
                            nc.scalar.activation(dst, p3[:, :, :, 0], AF.Copy)
                    return emit

                thunks = [make(j) for j in range(JH)]
                return wt, thunks

            wbt = []
            for hh in range(HH):
                wt, thunks = stage_transpose_ops(0, hh, wsgn0[hh])
                for t_ in thunks:
                    t_()
                wbt.append(wt)

            for og in range(OG):
                pending = []
                if og + 1 < OG:
                    nxt = [stage_load(og + 1, hh) for hh in range(HH)]
                    next_wbt = []
                    for hh in range(HH):
                        wt, thunks = stage_transpose_ops(og + 1, hh, nxt[hh])
                        next_wbt.append(wt)
                        pending.extend(thunks)
                n_mains = TB * HH * (JH // 2) * 2 * G
                stride = (max(1, (n_mains * 3 // 5) // max(1, len(pending)))
                          if pending else 0)
                mi = 0
                for tb in range(TB):
                    t0 = tb * P
                    pmg = [pmm.tile([P, ON], F32, tag="mm",
                                    name=f"pm_{og}_{tb}_{g}")
                           for g in range(G)]
                    osr = [None] * G

                    def emit_os(g, og=og, tb=tb, t0=t0, osr=osr):
                        # out_scale strip, staged through psum into an SBUF
                        # ring so the psum slot frees immediately
                        q0 = (og * G + g) * ON
                        osp = pTw.tile([P, ON], F32, tag="osps", name=f"osp_{og}_{tb}_{g}")
                        nc.tensor.matmul(
                            osp,
                            expT[:, t0 : t0 + P],
                            ocs_bf[:, q0 : q0 + ON],
                            start=True,
                            stop=True,
                        )
                        osr[g] = sb.tile([P, ON], BF16, tag="osr", bufs=4,
                                         name=f"osr_{og}_{tb}_{g}")
                        nc.scalar.activation(osr[g], osp, AF.Copy)

                    emit_os(0)
                    for hh in range(HH):
                        wt4 = wbt[hh].rearrange(
                            "p (jp two g n) -> p jp two g n",
                            two=2, g=G, n=ON,
                        )
                        for jp in range(JH // 2):
                            hp = hh * (JH // 2) + jp
                            for hl, A in ((0, xHi4), (1, xLo4)):
                                lhs = A[:, hp, :, t0 : t0 + P]
                                for g in range(G):
                                    nc.tensor.matmul(
                                        pmg[g],
                                        lhs,
                                        wt4[:, jp, :, g, :],
                                        start=(hh == 0 and jp == 0
                                               and hl == 0),
                                        stop=(hh == HH - 1
                                              and jp == JH // 2 - 1
                                              and hl == 1),
                                        perf_mode=DR,
                                    )
                                    mi += 1
                                    if pending and stride and mi % stride == 0:
                                        pending.pop(0)()
                        if hh == 0 or HH == 1:
                            emit_os(1)
                    for g in range(G):
                        oc = og * G + g
                        q0 = oc * ON
                        tmp = sb.tile([P, ON], F32, tag="out", bufs=2)
                        nc.vector.tensor_tensor(tmp, pmg[g], osr[g], ALU.mult)
                        nc.vector.tensor_tensor(
                            tmp, tmp, bias_bc[:, q0 : q0 + ON], ALU.add
                        )
                        nc.gpsimd.dma_start(
                            out=out_d[t0 : t0 + P, q0 : q0 + ON], in_=tmp
                        )
                for t_ in pending:
                    t_()
                if og + 1 < OG:
                    wbt = next_wbt
    return nc


_NC_CACHE = {}


def _get_nc(key=None):
    if key is None:
        key = (TOK, FULL_H, FULL_O)
    if key not in _NC_CACHE:
        _NC_CACHE[key] = build_nc(*key)
    return _NC_CACHE[key]


def kernel(x, weight, bias, gate_w, in_channel_scale, out_channel_scale):
    B, S, H = x.shape
    xf = np.ascontiguousarray(x.reshape(-1, H).astype(np.float32, copy=False))
    weight = np.ascontiguousarray(weight.astype(np.float32, copy=False))
    bias = np.ascontiguousarray(bias.astype(np.float32, copy=False))
    gate_w = np.ascontiguousarray(gate_w.astype(np.float32, copy=False))
    ics = np.ascontiguousarray(in_channel_scale.astype(np.float32, copy=False))
    ocs = np.ascontiguousarray(out_channel_scale.astype(np.float32, copy=False))

    nc = _get_nc()
    in_maps = [
        {
            "x": xf[c * TOK : (c + 1) * TOK],
            "weight": weight,
            "bias": bias,
            "gate_w": gate_w,
            "ics": ics,
            "ocs": ocs,
        }
        for c in range(N_CORES)
    ]
    res = run_bass_kernel_spmd(nc, in_maps, list(range(N_CORES)))
    out = np.concatenate(
        [res.results[c]["out"] for c in range(N_CORES)], axis=0
    )
    return out.reshape(B, S, -1)


# revision 3
# speedup vs baseline: 1.0455x; 1.0455x over previous
"""BinaryMoSLinear Trainium2 kernel, fp8-DoubleRow edition (8-core SPMD, DP over tokens).

Math (per reference):
    routing = softmax(xf @ gate_w.T);  in_s = routing @ ics;  out_s = routing @ ocs
    out = (xf * in_s) @ sign(weight).T * out_s + bias

Device factorization (per core: 1024 tokens, full weight):
    expT[e,t] = exp(logitsT[e,t]) raw; den = sum_e expT; is_raw = ics^T expT
    a[h,t] = xT * is_raw   (softmax denominators factored out, applied at the end)
    main[t,o] = a^T sign(w)^T;  out = main * os_raw / den^2 + bias

fp8 path: a is stored as an e4m3 hi/lo pair (aHi + aLo ~ bf16 accuracy); sign(w)
is exact in e4m3.  Mains run in DoubleRow perf mode: each matmul contracts TWO
128-deep h-chunks (stationary [128,2,128], moving [128,2,512]) at 0.5 cyc/row —
2x the bf16 rate for hi+lo combined.  The stationary (an a-pair) is reused
across G=2 o-chunks to amortize ldweights.
"""

import numpy as np

import concourse.bass as bass
import concourse.mybir as mybir
from concourse import tile
from concourse.bass_utils import run_bass_kernel_spmd
from concourse.masks import make_identity

F32 = mybir.dt.float32
BF16 = mybir.dt.bfloat16
FP8 = mybir.dt.float8e4
DR = mybir.MatmulPerfMode.DoubleRow
AF = mybir.ActivationFunctionType
ALU = mybir.AluOpType

P = 128
E = 8
N_CORES = 8

FULL_B, FULL_S, FULL_H, FULL_O = 4, 2048, 4096, 4096
TOK = FULL_B * FULL_S // N_CORES  # 1024 tokens per core

ON = 512      # psum / main moving width per o-chunk
G = 2         # o-chunks sharing one stationary load (wbt group width = G*ON)
WBT_BUFS = 6  # wbt stage ring


# --------------------------------------------------------------------------
# This container's walrus build accepts at most ONE sync-wait command per
# instruction.  Tile's scheduler freely stacks several waits on one
# instruction, so rewrite the BIR JSON before compile: excess waits become
# single-wait NoOps immediately preceding the instruction on the same engine.
_MAXW = 1


def _split_excess_waits(bir_json: bytes, maxw: int = _MAXW) -> bytes:
    import json as _json

    j = _json.loads(bir_json)
    ctr = 0
    for fn in j["functions"]:
        for blk in fn["blocks"]:
            new = []
            for inst in blk["instructions"]:
                si = inst.get("sync_info")
                if si:
                    waits = si.get("on_wait") or []
                    if len(waits) > maxw:
                        extra, keep = waits[:-maxw], waits[-maxw:]
                        for i in range(0, len(extra), maxw):
                            ctr += 1
                            nop = {
                                "name": f"I-wsplit-{ctr}",
                                "opcode": "NoOp",
                                "engine": inst["engine"],
                                "ins": [],
                                "outs": [],
                                "sync_info": {
                                    "on_wait": extra[i : i + maxw],
                                    "on_update": [],
                                },
                            }
                            if "debug" in inst:
                                nop["debug"] = inst["debug"]
                            new.append(nop)
                        si["on_wait"] = keep
                new.append(inst)
            blk["instructions"] = new
    return _json.dumps(j).encode()


def _install_wait_split():
    from concourse import bass2jax, bass_utils

    orig = bass_utils.compile_bir_kernel
    if getattr(orig, "_wait_split_wrapped", False):
        return

    def wrapped(bir_json, tmpdir, neff_name="file.neff"):
        return orig(_split_excess_waits(bir_json), tmpdir, neff_name)

    wrapped._wait_split_wrapped = True
    bass_utils.compile_bir_kernel = wrapped
    bass2jax.compile_bir_kernel = wrapped


_install_wait_split()
# --------------------------------------------------------------------------


def build_nc(tok=TOK, h=FULL_H, o=FULL_O):
    HC = h // P              # h chunks
    HP = HC // 2             # h chunk pairs (DoubleRow k-tiles)
    TB = tok // P            # token blocks
    THW = min(512, tok)      # gating token-half width
    TH = tok // THW
    OC = o // ON             # o chunks
    OG = OC // G             # o-chunk groups (stationary reuse)
    JH = min(8, HC)          # h-chunks per wbt stage
    HH = HC // JH            # stages per group
    NSG = G * ON // P        # o-strips per stage
    GON = G * ON
    assert tok % P == 0 and h % (2 * P) == 0 and o % GON == 0 and HC % JH == 0

    nc = bass.Bass("TRN2", target_bir_lowering=False, debug=False,
                   num_devices=N_CORES)

    x_d = nc.declare_dram_parameter("x", [tok, h], F32, isOutput=False)
    w_d = nc.declare_dram_parameter("weight", [o, h], F32, isOutput=False)
    b_d = nc.declare_dram_parameter("bias", [o], F32, isOutput=False)
    gw_d = nc.declare_dram_parameter("gate_w", [E, h], F32, isOutput=False)
    ics_d = nc.declare_dram_parameter("ics", [E, h], F32, isOutput=False)
    ocs_d = nc.declare_dram_parameter("ocs", [E, o], F32, isOutput=False)
    out_d = nc.declare_dram_parameter("out", [tok, o], F32, isOutput=True)

    with tile.TileContext(nc) as tc:
        with (
            tc.tile_pool(name="const", bufs=1) as const,
            tc.tile_pool(name="sb", bufs=2) as sb,
            tc.tile_pool(name="wsgn", bufs=16) as wsgnp,
            tc.tile_pool(name="wbt", bufs=WBT_BUFS) as wbtp,
            tc.tile_pool(name="tring", bufs=2) as tringp,
            tc.tile_pool(name="pmm", bufs=4, space="PSUM") as pmm,
            tc.tile_pool(name="pTx", bufs=2, space="PSUM") as pTx,
            tc.tile_pool(name="pTw", bufs=1, space="PSUM") as pTw,
        ):
            # ---- early DMA: first x strips + weight stage 0 ----
            x_bf_tiles = {}
            CH = min(JH * P, h)
            x_bf_tiles[0] = sb.tile([P, h], BF16, tag="xbf", bufs=4,
                                    name="xbf_pre0")
            for c0 in range(0, h, CH):
                xpre = sb.tile([P, JH * P], F32, tag="wf32",
                               name=f"xpre_{c0}")
                nc.sync.dma_start(
                    out=xpre[:, 0:CH], in_=x_d[0:P, c0 : c0 + CH]
                )
                nc.vector.tensor_copy(
                    out=x_bf_tiles[0][:, c0 : c0 + CH], in_=xpre[:, 0:CH]
                )
            aux1 = const.tile([P, max(h, o)], BF16, name="aux1")
            nc.gpsimd.dma_start(out=aux1[0:E, 0:h], in_=gw_d[:, :])
            if TB > 1:
                x_bf_tiles[1] = sb.tile([P, h], BF16, tag="xbf", bufs=4,
                                        name="xbf_pre1")
                nc.gpsimd.dma_start(out=x_bf_tiles[1], in_=x_d[P : 2 * P, :])
            if TB > 2:
                x_bf_tiles[2] = sb.tile([P, h], BF16, tag="xbf", bufs=4,
                                        name="xbf_pre2")
                nc.gpsimd.dma_start(out=x_bf_tiles[2], in_=x_d[2 * P : 3 * P, :])

            def stage_load(og, hh):
                """DMA + sign one weight stage: o-cols [og*GON, (og+1)*GON),
                h-cols [hh*JH*P, (hh+1)*JH*P). Returns fp8-signed strips."""
                o0 = og * GON
                wsgn = []
                for st in range(NSG):
                    wf = sb.tile([P, JH * P], F32, tag="wf32",
                                 name=f"wf_{og}_{hh}_{st}")
                    nc.sync.dma_start(
                        out=wf,
                        in_=w_d[
                            o0 + st * P : o0 + (st + 1) * P,
                            hh * JH * P : (hh + 1) * JH * P,
                        ],
                    )
                    ws = wsgnp.tile([P, JH * P], FP8, tag="wsgn",
                                    name=f"ws_{og}_{hh}_{st}")
                    nc.scalar.activation(ws, wf, AF.Sign)
                    wsgn.append(ws)
                return wsgn

            wsgn0 = [stage_load(0, hh) for hh in range(HH)]

            # ---- constants ----
            id_bf = const.tile([P, P], BF16, name="id_bf")
            make_identity(nc, id_bf)
            id8 = const.tile([P, P], FP8, name="id8")
            make_identity(nc, id8)
            ones_bf = const.tile([P, 1], BF16, name="ones_bf")
            nc.vector.memset(ones_bf, 1.0)
            ones_e = const.tile([P, E], F32, name="ones_e")
            nc.vector.memset(ones_e, 1.0)

            # aT holds bf16 xT, then (after the in_scale fold) its bytes are
            # reused in place: even bytes = fp8 hi, odd bytes = fp8 lo.
            aT = const.tile([P, HC * tok], BF16, name="aT")
            aT3 = aT.rearrange("p (hc t) -> p hc t", t=tok)
            _ab = aT.bitcast(FP8).rearrange("p (n two) -> p n two", two=2)
            xHi = _ab[:, :, 0]
            xLo = _ab[:, :, 1]
            xHi3 = xHi.rearrange("p (hc t) -> p hc t", t=tok)
            xLo3 = xLo.rearrange("p (hc t) -> p hc t", t=tok)
            xHi4 = xHi.rearrange("p (hp two t) -> p hp two t", two=2, t=tok)
            xLo4 = xLo.rearrange("p (hp two t) -> p hp two t", two=2, t=tok)

            expT = const.tile([P, tok], BF16, name="expT")
            nc.vector.memset(expT, 0.0)

            gwT = const.tile([P, HC * E], BF16, name="gwT")
            # aux1: holds gate_w for the gwT build, then is overwritten with
            # the broadcast bias for the epilogue.  aux2: in_channel_scale
            # rows during phase A, then out_channel_scale rows for phase C
            # (rows E.. are never read through a nonzero stationary).
            aux2 = const.tile([P, max(h, o)], BF16, name="aux2")
            nc.vector.memset(aux2, 0.0)
            nc.gpsimd.dma_start(out=aux2[0:E, 0:h], in_=ics_d[:, :])
            gw_bf = aux1
            ics_bf = aux2

            # gwT: transpose gate_w (E rows live, zero-padded)
            for hc in range(HC):
                pt = pTx.tile([P, min(8, HC) * P], BF16, tag="Tx",
                              name=f"ptg_{hc}")
                nc.tensor.transpose(
                    pt[:, 0:P], gw_bf[:, hc * P : (hc + 1) * P], id_bf
                )
                nc.vector.tensor_copy(
                    out=gwT[:, hc * E : (hc + 1) * E], in_=pt[:, 0:E]
                )
            bias_bc = aux1
            nc.gpsimd.dma_start(
                out=bias_bc[:, 0:o], in_=b_d[None, :].to_broadcast((P, o))
            )

            # ---- phase A1: transpose x into fp8 hi/lo ----
            for tb in range(TB):
                t0 = tb * P
                if tb in x_bf_tiles:
                    x_bf = x_bf_tiles.pop(tb)
                else:
                    x_bf = sb.tile([P, h], BF16, tag="xbf", bufs=4)
                    nc.gpsimd.dma_start(out=x_bf, in_=x_d[t0 : t0 + P, :])
                XG = min(8, HC)
                for gi, j0 in enumerate(range(0, HC, XG)):
                    pt = pTx.tile([P, XG * P], BF16, tag="Tx")
                    for k in range(XG):
                        nc.tensor.transpose(
                            pt[:, k * P : (k + 1) * P],
                            x_bf[:, (j0 + k) * P : (j0 + k + 1) * P],
                            id_bf,
                        )
                    ptv = pt.rearrange("p (b t) -> p b t", t=P)
                    dst = aT3[:, j0 : j0 + XG, t0 : t0 + P]
                    if gi % 2 == 0:
                        nc.scalar.activation(dst, ptv, AF.Copy)
                    else:
                        nc.vector.tensor_copy(out=dst, in_=ptv)

            # ---- phase C: weight transpose stages + DoubleRow mains ----
            def stage_transpose_ops(og, hh, wsgn):
                """Thunks: each transposes 4 o-strip blocks of one h-chunk j
                into the wbt stage tile (fp8, stride-2 psum staging)."""
                wt = wbtp.tile([P, JH * GON], FP8, tag="wbt",
                               name=f"wt_{og}_{hh}")

                def make(j):
                    def emit():
                        pt8 = pTw.tile([P, NSG * P * 2], FP8, tag="Tw",
                                       name=f"ptw_{og}_{hh}_{j}")
                        p3 = pt8.rearrange("p (b t two) -> p b t two",
                                           b=NSG, t=P)
                        for st in range(NSG):
                            nc.tensor.transpose(
                                p3[:, st, :, 0],
                                wsgn[st][:, j * P : (j + 1) * P],
                                id8,
                            )
                        eng = nc.vector if j % 2 == 0 else nc.scalar
                        dst = wt.rearrange("p (jj s) -> p jj s", s=GON)[
                            :, j, :
                        ].rearrange("p (b t) -> p b t", t=P)
                        if j % 2 == 0:
                            nc.vector.tensor_copy(out=dst, in_=p3[:, :, :, 0])
                        else:
                            nc.scalar.activation(dst, p3[:, :, :, 0], AF.Copy)
                    return emit

                thunks = [make(j) for j in range(JH)]
                return wt, thunks

            wbt = []
            thunks0 = []
            for hh in range(HH):
                wt, thunks = stage_transpose_ops(0, hh, wsgn0[hh])
                thunks0.extend(thunks)
                wbt.append(wt)


            # ---- phase A2: gating + fold + hi/lo re-split, per token half ----
            for th in range(TH):
                s0 = th * THW
                pl = pmm.tile([P, ON], F32, tag="mm", name=f"pl_{th}")
                for hc in range(HC):
                    nc.tensor.matmul(
                        pl[0:E, 0:THW],
                        gwT[:, hc * E : (hc + 1) * E],
                        aT3[:, hc, s0 : s0 + THW],
                        start=(hc == 0),
                        stop=(hc == HC - 1),
                    )
                nc.scalar.activation(
                    expT[0:E, s0 : s0 + THW], pl[0:E, 0:THW], AF.Exp
                )
                pd = pmm.tile([P, ON], F32, tag="mm")
                nc.tensor.matmul(
                    pd[0:1, 0:THW], ones_bf, expT[:, s0 : s0 + THW],
                    start=True, stop=True,
                )
                idr = sb.tile([P, ON], F32, tag="out", bufs=2,
                              name=f"idr_{th}")
                nc.vector.reciprocal(idr[0:1, 0:THW], pd[0:1, 0:THW])
                pde = pmm.tile([P, ON], F32, tag="mm")
                nc.tensor.matmul(
                    pde[0:E, 0:THW], ones_e[0:1, 0:E], idr[0:1, 0:THW],
                    start=True, stop=True,
                )
                nc.vector.tensor_tensor(
                    expT[0:E, s0 : s0 + THW],
                    expT[0:E, s0 : s0 + THW],
                    pde[0:E, 0:THW],
                    ALU.mult,
                )
                for hc in range(HC):
                    if thunks0:
                        thunks0.pop(0)()
                    pis = pmm.tile([P, ON], F32, tag="mm")
                    nc.tensor.matmul(
                        pis[:, 0:THW],
                        ics_bf[:, hc * P : (hc + 1) * P],
                        expT[:, s0 : s0 + THW],
                        start=True,
                        stop=True,
                    )
                    tmp = tringp.tile([P, THW], BF16, tag="t")
                    nc.vector.tensor_tensor(
                        tmp, aT3[:, hc, s0 : s0 + THW], pis[:, 0:THW],
                        ALU.mult,
                    )
                    nc.scalar.activation(
                        xHi3[:, hc, s0 : s0 + THW], tmp, AF.Copy
                    )
                    nc.vector.tensor_tensor(
                        xLo3[:, hc, s0 : s0 + THW],
                        tmp,
                        xHi3[:, hc, s0 : s0 + THW],
                        ALU.subtract,
                    )

            for t_ in thunks0:
                t_()
            thunks0 = []
            ocs_bf = aux2
            nc.gpsimd.dma_start(out=ocs_bf[0:E, 0:o], in_=ocs_d[:, :])

            for og in range(OG):
                pending = []
                if og + 1 < OG:
                    nxt = [stage_load(og + 1, hh) for hh in range(HH)]
                    next_wbt = []
                    for hh in range(HH):
                        wt, thunks = stage_transpose_ops(og + 1, hh, nxt[hh])
                        next_wbt.append(wt)
                        pending.extend(thunks)
                n_mains = TB * HH * (JH // 2) * 2 * G
                stride = (max(1, (n_mains * 3 // 5) // max(1, len(pending)))
                          if pending else 0)
                mi = 0
                for tb in range(TB):
                    t0 = tb * P
                    pmg = [pmm.tile([P, ON], F32, tag="mm",
                                    name=f"pm_{og}_{tb}_{g}")
                           for g in range(G)]
                    osr = [None] * G

                    def emit_os(g, og=og, tb=tb, t0=t0, osr=osr):
                        # out_scale strip, staged through psum into an SBUF
                        # ring so the psum slot frees immediately
                        q0 = (og * G + g) * ON
                        osp = pTw.tile([P, ON], F32, tag="osps", name=f"osp_{og}_{tb}_{g}")
                        nc.tensor.matmul(
                            osp,
                            expT[:, t0 : t0 + P],
                            ocs_bf[:, q0 : q0 + ON],
                            start=True,
                            stop=True,
                        )
                        osr[g] = sb.tile([P, ON], BF16, tag="osr", bufs=4,
                                         name=f"osr_{og}_{tb}_{g}")
                        nc.scalar.activation(osr[g], osp, AF.Copy)

                    emit_os(0)
                    for hh in range(HH):
                        wt4 = wbt[hh].rearrange(
                            "p (jp two g n) -> p jp two g n",
                            two=2, g=G, n=ON,
                        )
                        for jp in range(JH // 2):
                            hp = hh * (JH // 2) + jp
                            for hl, A in ((0, xHi4), (1, xLo4)):
                                lhs = A[:, hp, :, t0 : t0 + P]
                                for g in range(G):
                                    nc.tensor.matmul(
                                        pmg[g],
                                        lhs,
                                        wt4[:, jp, :, g, :],
                                        start=(hh == 0 and jp == 0
                                               and hl == 0),
                                        stop=(hh == HH - 1
                                              and jp == JH // 2 - 1
                                              and hl == 1),
                                        perf_mode=DR,
                                    )
                                    mi += 1
                                    if pending and stride and mi % stride == 0:
                                        pending.pop(0)()
                        if hh == 0 or HH == 1:
                            emit_os(1)
                    for g in range(G):
                        oc = og * G + g
                        q0 = oc * ON
                        tmp = sb.tile([P, ON], F32, tag="out", bufs=2)
                        nc.vector.tensor_tensor(tmp, pmg[g], osr[g], ALU.mult)
                        nc.vector.tensor_tensor(
                            tmp, tmp, bias_bc[:, q0 : q0 + ON], ALU.add
                        )
                        nc.gpsimd.dma_start(
                            out=out_d[t0 : t0 + P, q0 : q0 + ON], in_=tmp
                        )
                for t_ in pending:
                    t_()
                if og + 1 < OG:
                    wbt = next_wbt
    return nc


_NC_CACHE = {}


def _get_nc(key=None):
    if key is None:
        key = (TOK, FULL_H, FULL_O)
    if key not in _NC_CACHE:
        _NC_CACHE[key] = build_nc(*key)
    return _NC_CACHE[key]


def kernel(x, weight, bias, gate_w, in_channel_scale, out_channel_scale):
    B, S, H = x.shape
    xf = np.ascontiguousarray(x.reshape(-1, H).astype(np.float32, copy=False))
    weight = np.ascontiguousarray(weight.astype(np.float32, copy=False))
    bias = np.ascontiguousarray(bias.astype(np.float32, copy=False))
    gate_w = np.ascontiguousarray(gate_w.astype(np.float32, copy=False))
    ics = np.ascontiguousarray(in_channel_scale.astype(np.float32, copy=False))
    ocs = np.ascontiguousarray(out_channel_scale.astype(np.float32, copy=False))

    nc = _get_nc()
    in_maps = [
        {
            "x": xf[c * TOK : (c + 1) * TOK],
            "weight": weight,
            "bias": bias,
            "gate_w": gate_w,
            "ics": ics,
            "ocs": ocs,
        }
        for c in range(N_CORES)
    ]
    res = run_bass_kernel_spmd(nc, in_maps, list(range(N_CORES)))
    out = np.concatenate(
        [res.results[c]["out"] for c in range(N_CORES)], axis=0
    )
    return out.reshape(B, S, -1)
